# revision 31
# baseline (speedup 1.0000x reference)
"""Trainium2 Bass kernel for nn_ChainModel (neural 1-form chain classifier).

Computation (per edge e, graph b):
    mid = 0.5*(p0+p1); t = p1-p0
    h = relu(mid @ W1 + b1)                      [64]
    omega = (h @ W2 + b2) viewed [32, 16]
    X[c] = omega[c, :] . t                       [32]
    feats[b, c] = sum_{e in b} X[c]^2
    BN(train-stats) -> 3-layer classifier -> log_softmax

Device pipeline (edge-major, fp16 on-chip, fp32 PSUM):
    B:  PE matmul  chainsT_aug[33,128chunk].T @ Wfront[33,112] -> h_pre|t|beta
    C:  ACT relu -> h (fp16);  DVE copy t|beta -> fp16
    T:  DMA-xbar transpose h [128,128] (2 chunks) -> lhsT layout
    D:  PE matmul (row-tiled x2) hT[64,128].T @ W2[64,512] -> omega (f32 PSUM)
    V:  ACT copy omega -> fp16 SBUF
    E:  DVE tensor_tensor mul (2x fp16) omega * t-broadcast -> psi
    F:  DVE halving-tree adds over d (4 levels) -> X;  + beta
    G:  ACT square -> X^2
    H:  PE matmul run-indicators[128,R].T @ X2[128,32] -> per-run sums (f32)
Host: fold run sums into per-graph feats, BN + classifier + log_softmax (f64).

Sharding: contiguous edge ranges, E/8 per core; graphs straddling core or
chunk boundaries are handled by the host-side run->graph accumulation.
"""

import numpy as np
from contextlib import ExitStack

# ---------------- problem constants (hardcoded per contest rules) -----------
E_TOT = 524288
B_GRAPHS = 256
DIN = 16
HID = 64
C_STEPS = 32
NCLS = 10
EPS = 1e-5
NCORES = 8
EC = E_TOT // NCORES          # 65536 edges per core
CHUNK = 128                   # edges per chunk
NCHUNK = EC // CHUNK          # 512
SUPER = 8                     # chunks per super-tile
NSUPER = NCHUNK // SUPER      # 64
KAUG = 2 * DIN + 1            # 33: chains dims + ones row
WF_COLS = HID + DIN + C_STEPS  # 112: h_pre | t | beta

_BUILD_CACHE = {}


# tuning knobs (values picked via TimelineSim study + HW wall time)
CFG = {
    "super": 16,        # chunks per super-tile
    "om_chunks": 2,     # chunks per om psum tile
    "om_bufs": 2,       # om psum tile bufs
    "rsb_engine": "scalar",   # runsums psum->sbuf evac engine
    "l1_engine": "vector",    # tree level-1 engine
    "evac": "act",      # omega evac: "act" fp16 | "none" (mul from psum)
}


# ============================ device program ================================
def _build_program(runs_per_chunk, cfg=None):
    import concourse.bacc as bacc
    import concourse.mybir as mybir
    import concourse.tile as tile

    dt = mybir.dt
    AF = mybir.ActivationFunctionType
    ALU = mybir.AluOpType
    R = runs_per_chunk
    cfg = dict(CFG, **(cfg or {}))
    SUPER = cfg["super"]
    NSUPER = NCHUNK // SUPER

    nc = bacc.Bacc("TRN2", target_bir_lowering=False, debug=False,
                   num_devices=NCORES)

    # DRAM I/O
    d_chains = nc.dram_tensor("chainsT", [KAUG, EC], dt.float16,
                              kind="ExternalInput").ap()
    d_wfront = nc.dram_tensor("wfront", [KAUG, WF_COLS], dt.float16,
                              kind="ExternalInput").ap()
    d_w2 = nc.dram_tensor("w2sb", [128, C_STEPS * DIN], dt.float16,
                          kind="ExternalInput").ap()
    d_ind = nc.dram_tensor("ind", [128, NCHUNK * R], dt.float16,
                           kind="ExternalInput").ap()
    d_out = nc.dram_tensor("runsums", [R, NCHUNK * C_STEPS], dt.float32,
                           kind="ExternalOutput").ap()
    if cfg.get("tpose") == "pe":
        d_ident = nc.dram_tensor("ident", [128, 128], dt.float16,
                                 kind="ExternalInput").ap()

    with tile.TileContext(nc) as tc, ExitStack() as ctx:
        const_pool = ctx.enter_context(tc.tile_pool(name="const", bufs=1))
        sb = ctx.enter_context(tc.tile_pool(name="sb", bufs=2))
        ps_b = ctx.enter_context(tc.tile_pool(name="psb", bufs=2, space="PSUM"))
        ps_o = ctx.enter_context(tc.tile_pool(
            name="pso", bufs=cfg["om_bufs"], space="PSUM"))
        ps_r = ctx.enter_context(tc.tile_pool(
            name="psr", bufs=cfg.get("rps_bufs", 1), space="PSUM"))
        if cfg.get("tpose") == "pe":
            ps_t = ctx.enter_context(tc.tile_pool(
                name="pst", bufs=1, space="PSUM"))

        # constants loaded once
        wf = const_pool.tile([KAUG, WF_COLS], dt.float16)
        nc.sync.dma_start(wf[:], d_wfront)
        w2 = const_pool.tile([128, C_STEPS * DIN], dt.float16)
        nc.sync.dma_start(w2[:], d_w2)
        ind = const_pool.tile([128, NCHUNK * R], dt.float16)
        nc.sync.dma_start(ind[:], d_ind)
        if cfg.get("tpose") == "pe":
            ident = const_pool.tile([128, 128], dt.float16)
            nc.sync.dma_start(ident[:], d_ident)

        for s in range(NSUPER):
            # ---- load chains super-tile [33, SUPER*CHUNK] ----
            ch = sb.tile([KAUG, SUPER * CHUNK], dt.float16, tag="ch")
            nc.sync.dma_start(ch[:], d_chains[:, s * SUPER * CHUNK:
                                             (s + 1) * SUPER * CHUNK])

            h_t = sb.tile([128, SUPER, HID], dt.float16, tag="h")
            tb = sb.tile([128, SUPER, DIN + C_STEPS], dt.float16, tag="tb")
            psi = sb.tile([128, SUPER, C_STEPS, DIN], dt.float16, tag="psi")

            if cfg.get("bprime"):
                # B': c-major h straight from the matmul (wf is the
                # stationary operand) -- no h transposes needed for D.
                h_cm = sb.tile([64, SUPER, CHUNK], dt.float16, tag="hcm")
                for g in range(SUPER // 4):
                    bpc = ps_b.tile([128, 4, CHUNK], dt.float32, tag="bps")
                    for j in range(4):
                        k = g * 4 + j
                        nc.tensor.matmul(
                            bpc[0:WF_COLS, j, :], wf[:],
                            ch[:, k * CHUNK:(k + 1) * CHUNK],
                            start=True, stop=True)
                    nc.scalar.activation(h_cm[:, g * 4:(g + 1) * 4, :],
                                         bpc[0:HID, :, :], AF.Relu)
                    # t|beta: c-major copy then xbar-transpose to edge-major
                    tbc = sb.tile([WF_COLS - HID, 4, CHUNK], dt.float16,
                                  tag="tbc")
                    nc.scalar.activation(tbc[:], bpc[HID:WF_COLS, :, :],
                                         AF.Copy)
                    for j in range(4):
                        k = g * 4 + j
                        nc.sync.dma_start_transpose(
                            tb[:, k, :], tbc[:, j, :])
            else:
                # ---- B: 4-chunk PSUM tiles [128, 4, 128] f32 (1 bank) ----
                # per-chunk slot padded to 128 f32 (512B) so no matmul
                # output crosses a 2 KiB PSUM bank boundary
                for g in range(SUPER // 4):
                    bps = ps_b.tile([128, 4, 128], dt.float32, tag="bps")
                    for j in range(4):
                        k = g * 4 + j
                        nc.tensor.matmul(
                            bps[:, j, 0:WF_COLS],
                            ch[:, k * CHUNK:(k + 1) * CHUNK],
                            wf[:],
                            start=True, stop=True)
                    # C: relu + t|beta copy
                    nc.scalar.activation(h_t[:, g * 4:(g + 1) * 4, :],
                                         bps[:, :, 0:HID], AF.Relu)
                    nc.vector.tensor_copy(tb[:, g * 4:(g + 1) * 4, :],
                                          bps[:, :, HID:WF_COLS])

            if cfg.get("ablate") in ("noD",):
                continue
            # ---- T + D + V + E per om-group ----
            OMC = cfg.get("om_chunks", 4)   # chunks per om psum tile (2 or 4)
            for q in range(SUPER // OMC):
                om = ps_o.tile([128, OMC, C_STEPS, DIN], dt.float32, tag="om")
                if cfg.get("bprime"):
                    for j in range(OMC):
                        k = q * OMC + j
                        nc.tensor.matmul(om[:, j, :, :], h_cm[:, k, :],
                                         w2[0:HID, :], start=True, stop=True)
                for j in range(OMC // 2) if not cfg.get("bprime") else []:
                    c0 = q * OMC + 2 * j    # chunks (c0, c0+1)
                    htt = sb.tile([128, 128], dt.float16, tag="htt")
                    if cfg.get("tpose") == "pe":
                        htp = ps_t.tile([128, 128], dt.float16, tag="htp")
                        nc.tensor.transpose(htp[:], h_t[:, c0:c0 + 2, :],
                                            ident[:])
                        nc.vector.tensor_copy(htt[:], htp[:])
                    elif cfg.get("tpose") == "alt":
                        # alternate HWDGE dispatch queues (SP / Activation)
                        eng = nc.sync if (q + j) % 2 == 0 else nc.scalar
                        eng.dma_start_transpose(
                            htt[:], h_t[:, c0:c0 + 2, :])
                    else:
                        nc.sync.dma_start_transpose(
                            htt[:], h_t[:, c0:c0 + 2, :])
                    nc.tensor.matmul(om[:, 2 * j, :, :], htt[0:HID, :],
                                     w2[0:HID, :], start=True, stop=True,
                                     tile_position=(0, 0))
                    nc.tensor.matmul(om[:, 2 * j + 1, :, :], htt[HID:128, :],
                                     w2[HID:128, :], start=True, stop=True,
                                     tile_position=(64, 0))
                t_in = tb[:, q * OMC:(q + 1) * OMC, 0:DIN]    # [128,OMC,16]
                t_b = t_in.unsqueeze(2).broadcast_to(
                    [128, OMC, C_STEPS, DIN])
                if cfg.get("ablate") in ("noV",):
                    continue
                if cfg["evac"] == "act":
                    # V: omega f32 PSUM -> fp16 SBUF (ScalarE, near PSUM)
                    oms = sb.tile([128, OMC, C_STEPS, DIN], dt.float16,
                                  tag="oms")
                    nc.scalar.activation(oms[:], om[:], AF.Copy)
                    mul_src = oms[:]
                else:
                    mul_src = om[:]
                if cfg.get("ablate") in ("noE",):
                    continue
                # E: psi = omega * t (broadcast t over cochains)
                nc.vector.tensor_tensor(
                    psi[:, q * OMC:(q + 1) * OMC, :, :], mul_src, t_b,
                    ALU.mult)

            if cfg.get("ablate") in ("noF", "noE", "noV"):
                continue
            # ---- F: halving tree over d + beta ----
            l1_eng = nc.gpsimd if cfg["l1_engine"] == "gpsimd" else nc.vector
            l1 = sb.tile([128, SUPER, C_STEPS, DIN // 2], dt.float16, tag="l1")
            if cfg.get("l1_split"):
                hs = SUPER // 2
                nc.vector.tensor_tensor(l1[:, 0:hs], psi[:, 0:hs, :, 0:8],
                                        psi[:, 0:hs, :, 8:16], ALU.add)
                nc.gpsimd.tensor_tensor(l1[:, hs:SUPER],
                                        psi[:, hs:SUPER, :, 0:8],
                                        psi[:, hs:SUPER, :, 8:16], ALU.add)
            else:
                l1_eng.tensor_tensor(l1[:], psi[:, :, :, 0:8],
                                     psi[:, :, :, 8:16], ALU.add)
            l2 = sb.tile([128, SUPER, C_STEPS, DIN // 4], dt.float16, tag="l2")
            nc.vector.tensor_tensor(l2[:], l1[:, :, :, 0:4],
                                    l1[:, :, :, 4:8], ALU.add)
            l3 = sb.tile([128, SUPER, C_STEPS, DIN // 8], dt.float16, tag="l3")
            nc.vector.tensor_tensor(l3[:], l2[:, :, :, 0:2],
                                    l2[:, :, :, 2:4], ALU.add)
            x4 = sb.tile([128, SUPER, C_STEPS], dt.float16, tag="x4")
            nc.vector.tensor_tensor(x4[:].unsqueeze(3), l3[:, :, :, 0:1],
                                    l3[:, :, :, 1:2], ALU.add)
            xf = sb.tile([128, SUPER, C_STEPS], dt.float32, tag="xf")
            nc.vector.tensor_tensor(xf[:], x4[:],
                                    tb[:, :, DIN:DIN + C_STEPS], ALU.add)

            # ---- G: square ----
            x2 = sb.tile([128, SUPER, C_STEPS], dt.float16, tag="x2")
            nc.scalar.activation(x2[:], xf[:], AF.Square)

            if cfg.get("ablate") in ("noH",):
                continue
            # ---- H: per-run segment sums ----
            rps = ps_r.tile([R, SUPER * C_STEPS], dt.float32, tag="rps")
            for k in range(SUPER):
                gk = s * SUPER + k
                nc.tensor.matmul(
                    rps[:, k * C_STEPS:(k + 1) * C_STEPS],
                    ind[:, gk * R:(gk + 1) * R],
                    x2[:, k, :],
                    start=True, stop=True)
            rsb = sb.tile([R, SUPER * C_STEPS], dt.float32, tag="rsb")
            if cfg["rsb_engine"] == "scalar":
                nc.scalar.activation(rsb[:], rps[:], AF.Copy)
            else:
                nc.vector.tensor_copy(rsb[:], rps[:])
            nc.sync.dma_start(
                d_out[:, s * SUPER * C_STEPS:(s + 1) * SUPER * C_STEPS],
                rsb[:])

    nc.finalize()
    return nc


def _get_program(runs_per_chunk, cfg=None):
    key = (runs_per_chunk, tuple(sorted((cfg or {}).items())))
    if key not in _BUILD_CACHE:
        _BUILD_CACHE[key] = _build_program(runs_per_chunk, cfg)
    return _BUILD_CACHE[key]


# ============================ host-side glue ================================
def _host_prep_weights(W1, b1, W2, b2):
    """Wfront [33,112] = mid-fold(W1) | diff | beta-fold(b2); w2sb [128,512]."""
    wf = np.zeros((KAUG, WF_COLS), np.float64)
    # h_pre cols: mid @ W1 = 0.5*p0@W1 + 0.5*p1@W1
    wf[0:DIN, 0:HID] = 0.5 * W1
    wf[DIN:2 * DIN, 0:HID] = 0.5 * W1
    wf[2 * DIN, 0:HID] = b1          # ones-row -> +b1
    # t cols: t = p1 - p0
    for d in range(DIN):
        wf[d, HID + d] = -1.0
        wf[DIN + d, HID + d] = 1.0
    # beta cols: beta[c] = sum_d t_d * b2[c*16+d]
    b2r = b2.reshape(C_STEPS, DIN)   # [c, d]
    for c in range(C_STEPS):
        wf[0:DIN, HID + DIN + c] = -b2r[c]
        wf[DIN:2 * DIN, HID + DIN + c] = b2r[c]
    w2sb = np.concatenate([W2, W2], axis=0)  # [128, 512]
    return wf.astype(np.float16), w2sb.astype(np.float16)


def _host_prep_core(chains_core):
    """chains slice [EC, 2, 16] -> chainsT_aug [33, EC] fp16."""
    flat = np.ascontiguousarray(chains_core.reshape(EC, 2 * DIN).T)  # [32, EC]
    out = np.empty((KAUG, EC), np.float16)
    out[0:2 * DIN] = flat.astype(np.float16)
    out[2 * DIN] = 1.0
    return out


def _host_runs(edge_slices):
    """Per-core run indicators + run->graph map.

    Returns R, and per core: ind [128, NCHUNK*R] fp16,
    run2graph [NCHUNK, R] int32 (-1 = unused).
    """
    es = np.asarray(edge_slices, np.int64)
    seg = np.searchsorted(es, np.arange(E_TOT), side="right") - 1
    seg = np.clip(seg, 0, B_GRAPHS - 1).astype(np.int32)

    segc = seg.reshape(NCORES * NCHUNK, CHUNK)
    nruns = 1 + (np.diff(segc, axis=1) != 0).sum(axis=1)
    runs_needed = int(nruns.max())
    R = max(4, int(2 ** np.ceil(np.log2(runs_needed))))

    inds, maps = [], []
    for core in range(NCORES):
        ind = np.zeros((128, NCHUNK * R), np.float16)
        r2g = np.full((NCHUNK, R), -1, np.int32)
        sc = seg[core * EC:(core + 1) * EC].reshape(NCHUNK, CHUNK)
        for k in range(NCHUNK):
            g = sc[k]
            bnd = np.flatnonzero(np.diff(g)) + 1
            starts = np.concatenate(([0], bnd))
            ends = np.concatenate((bnd, [CHUNK]))
            for r, (a, b) in enumerate(zip(starts, ends)):
                ind[a:b, k * R + r] = 1.0
                r2g[k, r] = g[a]
        inds.append(ind)
        maps.append(r2g)
    return R, inds, maps


def _host_tail(feats, gamma, beta, C1w, C1b, C2w, C2b, C3w, C3b):
    f = feats.astype(np.float64)
    mean = f.mean(axis=0)
    var = f.var(axis=0)
    f = (f - mean) / np.sqrt(var + EPS) * gamma.astype(np.float64) \
        + beta.astype(np.float64)
    h1 = np.maximum(f @ C1w.astype(np.float64) + C1b, 0.0)
    h2 = np.maximum(h1 @ C2w.astype(np.float64) + C2b, 0.0)
    logits = h2 @ C3w.astype(np.float64) + C3b
    mx = logits.max(axis=1, keepdims=True)
    lse = np.log(np.exp(logits - mx).sum(axis=1, keepdims=True)) + mx
    return (logits - lse).astype(np.float32)


def kernel(chains, W1, b1, W2, b2, gamma, beta,
           C1w, C1b, C2w, C2b, C3w, C3b, edge_slices,
           _trace=False):
    import concourse.bass_utils as bass_utils

    chains = np.asarray(chains, np.float32)
    R, inds, r2g = _host_runs(edge_slices)
    wf, w2sb = _host_prep_weights(np.asarray(W1, np.float64),
                                  np.asarray(b1, np.float64),
                                  np.asarray(W2, np.float64),
                                  np.asarray(b2, np.float64))

    nc = _get_program(R)
    in_maps = []
    for core in range(NCORES):
        m = {
            "chainsT": _host_prep_core(chains[core * EC:(core + 1) * EC]),
            "wfront": wf,
            "w2sb": w2sb,
            "ind": inds[core],
        }
        if CFG.get("tpose") == "pe":
            m["ident"] = np.eye(128, dtype=np.float16)
        in_maps.append(m)

    import time as _time
    _t0 = _time.time()
    try:
        res = bass_utils.run_bass_kernel_spmd(
            nc, in_maps, core_ids=list(range(NCORES)), trace=_trace)
    except Exception:
        # transient device errors (e.g. NRT_EXEC_UNIT_UNRECOVERABLE left by
        # a previous crashed process) usually clear on the next attempt
        _time.sleep(2.0)
        res = bass_utils.run_bass_kernel_spmd(
            nc, in_maps, core_ids=list(range(NCORES)), trace=_trace)
    kernel._last_run_s = _time.time() - _t0

    # fold run sums into per-graph feats
    feats = np.zeros((B_GRAPHS, C_STEPS), np.float64)
    for core in range(NCORES):
        rs = res.results[core]["runsums"].astype(np.float64)
        rs = rs.reshape(R, NCHUNK, C_STEPS)          # [r, chunk, c]
        m = r2g[core]                                 # [chunk, R]
        valid = m >= 0
        np.add.at(feats, m[valid],
                  np.transpose(rs, (1, 0, 2))[valid])

    out = _host_tail(feats, np.asarray(gamma), np.asarray(beta),
                     np.asarray(C1w), np.asarray(C1b),
                     np.asarray(C2w), np.asarray(C2b),
                     np.asarray(C3w), np.asarray(C3b))
    kernel._last_exec_ns = res.exec_time_ns
    return out
